# revision 34
# baseline (speedup 1.0000x reference)
"""GCMC graph-conv kernel for Trainium2, distributed over 8 NeuronCores.

Computes: agg = segment_sum((src_feats @ W.T + b) * cj [edge_src], edge_dst) * ci

Strategy (dst-sharded, one NEFF SPMD on 8 cores):
  - Each core owns 12500 destination nodes and the edges pointing to them.
  - Phase A: each core computes wh = (X_shard @ W.T + b) * cj_shard on the
    TensorEngine and writes it (bf16) into a packed table shard: each 256B row
    holds FOUR nodes' 32-feature messages (node prow -> row prow//4, subcol
    prow%4). Packing keeps dma_gather rows at the required 256B multiple while
    the whole 25088-row table stays addressable by int16 gather indices.
  - AllGather the 8 compact shards (0.8MB each) -> full table in every HBM.
  - Phase B: edges are bucketed by (dst block, q=prow%4, dst half). Each
    128-slot tile gathers its edges' table rows (dma_gather), builds a
    one-hot matrix over its 64-dst window (is_equal on VectorE), and
    scatter-sums via PSUM-accumulating matmuls (rhs = gathered columns
    [32q : 32q+32]). Scale by ci, DMA out.

All control structure (tile counts, windows) is common across the 8 cores
(max over cores); cores pad their slots (dst_shift=-1 kills the one-hot
column; gather idx 0 is harmless).
"""
import sys

if "/opt/trn_rl_repo" not in sys.path:
    sys.path.insert(0, "/opt/trn_rl_repo")

import numpy as np
import ml_dtypes

import concourse.bacc as bacc
import concourse.mybir as mybir
import concourse.tile as tile
from concourse.bass_utils import run_bass_kernel_spmd

# problem constants (hardcoded per harness contract)
N_NODES = 100000
N_EDGES = 1_600_000
IN_DIM = 128
OUT_DIM = 32
N_CORES = 8
SHARD = N_NODES // N_CORES          # 12500 dst nodes per core
NBLK = (SHARD + 127) // 128         # 98 dst blocks per core
SPAD = NBLK * 128                   # 12544 padded shard nodes
TROWS = SPAD * N_CORES // 4         # 25088 packed table rows (4 nodes each)
LROWS = SPAD // 4                   # 3136 packed rows per core shard
ROWELEM = 128                       # bf16 elems per table row = 256B
WIN = 128                           # one-hot window: full block (PSUM base 0)
GRP = 16                            # tiles per is_equal op
BB = 5                              # dst blocks per double-buffered batch
GCAP = 25                           # tiles per dma_gather call
ACH = 14                            # phase-A node blocks per chunk (98 = 7*14)
GKIND = "hbm"                       # gather source: "sbuf" | "hbm"
NSWQ = 2                            # SWDGE queues for gather overlap

F32 = mybir.dt.float32
BF16 = mybir.dt.bfloat16
I16 = mybir.dt.int16


def _plan(edge_src, edge_dst):
    """Pack edges into the common SPMD structure.

    Tiles are ordered per batch, q-major: for q in 0..4, for b in batch,
    the (b, q) bucket's tiles. This makes each batch's same-q tiles
    contiguous so the SBUF-gather path can stream-transpose per q-slab.

    meta:
      ntiles       total tiles
      q_of[t]      table subcolumn (edge prow % 4), global tile index
      batches      list of dicts: t0, tcnt, qn[4], btiles{b: [gi, ...]}
    per core:
      idx  [128, ntiles*8] int16  wrapped packed-row gather indices
      dst  [128, ntiles]   bf16   per-slot dst index in 128-window (-1 = pad)
    """
    src = np.asarray(edge_src).astype(np.int64)
    dst = np.asarray(edge_dst).astype(np.int64)

    core = dst // SHARD
    dst_loc = dst % SHARD
    blk = dst_loc // 128
    dib = dst_loc % 128
    prow = (src // SHARD) * SPAD + (src % SHARD)
    row = prow // 4
    q = prow % 4

    key = ((core * NBLK + blk) * 4 + q)
    order = np.argsort(key, kind="stable")
    s_key, s_dib, s_row = key[order], dib[order], row[order]

    n_cells = N_CORES * NBLK * 4
    bounds = np.searchsorted(s_key, np.arange(n_cells + 1))
    # tiles per (b, q) bucket: max over cores
    ntile_bq = np.zeros((NBLK, 4), np.int64)
    for b in range(NBLK):
        for kq in range(4):
            mx = 0
            for c in range(N_CORES):
                cid = (c * NBLK + b) * 4 + kq
                mx = max(mx, int(bounds[cid + 1] - bounds[cid]))
            ntile_bq[b, kq] = (mx + 127) // 128

    ntiles = 0
    q_of = []
    batches = []
    idx_cols = [[] for _ in range(N_CORES)]
    sh_cols = [[] for _ in range(N_CORES)]

    for b0 in range(0, NBLK, BB):
        b1 = min(b0 + BB, NBLK)
        t0 = ntiles
        qn = []
        btiles = {b: [] for b in range(b0, b1)}
        for kq in range(4):
            nq = 0
            for b in range(b0, b1):
                segs = []
                for c in range(N_CORES):
                    cid = (c * NBLK + b) * 4 + kq
                    segs.append((int(bounds[cid]), int(bounds[cid + 1])))
                for t in range(int(ntile_bq[b, kq])):
                    btiles[b].append(ntiles - t0)
                    ntiles += 1
                    nq += 1
                    q_of.append(kq)
                    for c in range(N_CORES):
                        s, e = segs[c]
                        p = s + t * 128
                        take = max(0, min(e - p, 128))
                        col_i = np.zeros(128, np.int16)
                        col_s = np.full(128, -1.0, np.float32)
                        if take > 0:
                            col_i[:take] = s_row[p:p + take]
                            col_s[:take] = s_dib[p:p + take]
                        idx_cols[c].append(col_i)
                        sh_cols[c].append(col_s)
            qn.append(nq)
        batches.append({"b0": b0, "b1": b1, "t0": t0, "tcnt": ntiles - t0,
                        "qn": qn, "btiles": btiles})

    meta = {"ntiles": ntiles, "q_of": q_of, "batches": batches}

    per_core = []
    for c in range(N_CORES):
        icols = np.stack(idx_cols[c], 0)          # [nt, 128]
        scols = np.stack(sh_cols[c], 0)           # [nt, 128]
        w = icols.reshape(ntiles, 8, 16).transpose(2, 0, 1).reshape(16, ntiles * 8)
        per_core.append({
            "idx": np.tile(w.astype(np.int16), (8, 1)),
            "dst": scols.T.astype(ml_dtypes.bfloat16),
        })
    return meta, per_core


def _phasea_perm():
    """Phase-A node order is identity: tile t, partition p holds local node
    128t + p = packed row index; table write offset is affine (64B * p)."""
    return np.arange(SPAD)


def _build(meta, mode="full", n_devices=N_CORES, no_cc=False, reps=1,
           gkind=GKIND):
    ntiles = meta["ntiles"]
    q_of = meta["q_of"]
    batches = meta["batches"]

    nc = bacc.Bacc("TRN2", target_bir_lowering=False, debug=False,
                   enable_asserts=True, num_devices=n_devices,
                   num_swdge_queues=NSWQ)

    xT = nc.dram_tensor("xT", [128, SPAD], F32, kind="ExternalInput")
    wT = nc.dram_tensor("wT", [128, OUT_DIM], F32, kind="ExternalInput")
    brep = nc.dram_tensor("brep", [128, OUT_DIM], F32, kind="ExternalInput")
    cjT = nc.dram_tensor("cjT", [128, NBLK], F32, kind="ExternalInput")
    ciT = nc.dram_tensor("ciT", [128, NBLK], F32, kind="ExternalInput")
    idx_d = nc.dram_tensor("idx", [128, ntiles * 8], I16, kind="ExternalInput")
    dst_d = nc.dram_tensor("dst", [128, ntiles], BF16, kind="ExternalInput")
    out = nc.dram_tensor("out", [SPAD, OUT_DIM], F32, kind="ExternalOutput")

    gmax = max(bt["tcnt"] for bt in batches)

    with tile.TileContext(nc) as tc:
        with (
            tc.tile_pool(name="dram", bufs=1, space="DRAM") as dram,
            tc.tile_pool(name="const", bufs=1) as cpool,
            tc.tile_pool(name="xa", bufs=2) as xpool,
            tc.tile_pool(name="ha", bufs=4) as hpool,
            tc.tile_pool(name="wa", bufs=2) as wpool,
            tc.tile_pool(name="pa", bufs=4, space="PSUM") as ppa,
            tc.tile_pool(name="gath", bufs=2) as gpool,
            tc.tile_pool(name="idxp", bufs=2) as ipool,
            tc.tile_pool(name="msgp", bufs=2) as mpool,
            tc.tile_pool(name="smat", bufs=2) as spool,
            tc.tile_pool(name="pb", bufs=4, space="PSUM") as ppb,
            tc.tile_pool(name="res", bufs=2) as rpool,
        ):
            table_loc = dram.tile([LROWS, ROWELEM], BF16)
            table_full = dram.tile([TROWS, ROWELEM], BF16)

            # constants
            wt_t = cpool.tile([128, OUT_DIM], F32)
            nc.sync.dma_start(out=wt_t[:], in_=wT[:])
            br_t = cpool.tile([128, OUT_DIM], F32)
            nc.sync.dma_start(out=br_t[:], in_=brep[:])
            b1 = br_t[0:1, :]                   # bias as 1-partition rhs
            ones1 = cpool.tile([1, 128], F32)
            nc.vector.memset(ones1[:], 1.0)
            cj_t = cpool.tile([128, NBLK], F32)
            nc.sync.dma_start(out=cj_t[:], in_=cjT[:])
            ci_t = cpool.tile([128, NBLK], F32)
            nc.sync.dma_start(out=ci_t[:], in_=ciT[:])
            dst_t = cpool.tile([128, ntiles], BF16)
            nc.sync.dma_start(out=dst_t[:], in_=dst_d[:])
            # iota: [128, GRP*WIN] bf16, value = col % WIN
            io_i = cpool.tile([128, GRP * WIN], I16)
            nc.gpsimd.iota(io_i[:], pattern=[[0, GRP], [1, WIN]], base=0,
                           channel_multiplier=0)
            io_b = cpool.tile([128, GRP * WIN], BF16)
            nc.vector.tensor_copy(out=io_b[:], in_=io_i[:])
            gsem = nc.alloc_semaphore("gsem")
            if gkind == "sbuf":
                tbl_sb = cpool.tile([128, TROWS // 128, ROWELEM], BF16)

            # packed-table write view: chunk c, partition p, block j, feat f
            # -> DRAM offset 8192*(ACH*c + j) + 64*p + 2*f  (bytes; affine)
            tab_w = table_loc[:].rearrange("(c j r) (q f) -> c (r q) j f",
                                           j=ACH, r=32, q=4)

            for _rep in range(reps):
                # ---- Phase A: wh = (X @ W.T + b) * cj -> packed bf16 shard ----
                ntile_a = SPAD // 128  # 98
                nchunk = ntile_a // ACH
                for c in range(nchunk):
                    a0 = c * ACH
                    xt = xpool.tile([128, ACH * 128], F32)
                    nc.sync.dma_start(out=xt[:],
                                      in_=xT[:, a0 * 128:(a0 + ACH) * 128])
                    wh = wpool.tile([128, ACH, OUT_DIM], BF16)
                    for j in range(ACH):
                        ph = ppa.tile([128, OUT_DIM], F32, space="PSUM")
                        # bias via rank-1 accumulating matmul, then X @ W.T
                        nc.tensor.matmul(out=ph[:], lhsT=ones1[:], rhs=b1[:],
                                         start=True, stop=False,
                                         skip_group_check=True)
                        nc.tensor.matmul(out=ph[:], lhsT=xt[:, j * 128:(j + 1) * 128],
                                         rhs=wt_t[:], start=False, stop=True,
                                         skip_group_check=True)
                        nc.scalar.mul(out=wh[:, j, :], in_=ph[:],
                                      mul=cj_t[:, a0 + j:a0 + j + 1])
                    nc.sync.dma_start(out=tab_w[c], in_=wh[:])

                # ---- AllGather compact table shards ----
                if mode != "A" and not no_cc:
                    nc.gpsimd.collective_compute(
                        "AllGather",
                        mybir.AluOpType.bypass,
                        replica_groups=[list(range(N_CORES))],
                        ins=[table_loc.opt()],
                        outs=[table_full.opt()],
                    )

                # ---- table DRAM -> SBUF stripes (SBUF-source gather) ----
                if gkind == "sbuf" and mode not in ("A", "AG"):
                    nc.sync.dma_start(
                        out=tbl_sb[:],
                        in_=table_full[:].rearrange("(s p) f -> p s f", p=128))

                # ---- Phase B ----
                for bt in batches:
                    b0, b1, t0, tcnt = bt["b0"], bt["b1"], bt["t0"], bt["tcnt"]
                    s = spool.tile([128, gmax * WIN], BF16, tag="s")
                    if gkind == "sbuf":
                        g = gpool.tile([128, 1, gmax * 128], BF16, tag="g")
                        msg = mpool.tile([128, gmax, OUT_DIM], BF16, tag="m")
                    else:
                        g = gpool.tile([128, gmax, ROWELEM], BF16, tag="g")
                    if mode not in ("A", "AG"):
                        idx_t = ipool.tile([128, gmax * 8], I16, tag="i")
                        nc.sync.dma_start(
                            out=idx_t[:, 0:tcnt * 8],
                            in_=idx_d[:, t0 * 8:(t0 + tcnt) * 8])
                        for c0 in range(0, tcnt, GCAP):
                            cn = min(GCAP, tcnt - c0)
                            isl = idx_t[:, c0 * 8:(c0 + cn) * 8]
                            if gkind == "sbuf":
                                nc.gpsimd.dma_gather(
                                    out_ap=g[:, :, c0 * 128:(c0 + cn) * 128],
                                    in_ap=tbl_sb[:].rearrange("p s f -> p (s f)"),
                                    idxs_ap=isl,
                                    num_idxs=cn * 128, num_idxs_reg=cn * 128,
                                    elem_size=ROWELEM, transpose=True,
                                    sbuf_tokens_per_rank=128,
                                    sbuf_free_dim_per_rank=2 * ROWELEM,
                                )
                            else:
                                nc.gpsimd.dma_gather(
                                    out_ap=g[:, c0:c0 + cn, :],
                                    in_ap=table_full[:],
                                    idxs_ap=isl,
                                    num_idxs=cn * 128, num_idxs_reg=cn * 128,
                                    elem_size=ROWELEM, single_packet=False,
                                    queue_num=(c0 // GCAP) % NSWQ,
                                )
                        if gkind == "sbuf" and mode not in ("G",):
                            # per-q stream-transpose: msgT [32, E] -> msg [E, 32]
                            off = 0
                            for kq in range(4):
                                nq = bt["qn"][kq]
                                if nq == 0:
                                    continue
                                gq = g[32 * kq:32 * kq + 32, 0,
                                       off * 128:(off + nq) * 128] \
                                    .rearrange("p (t r f) -> p t r f", r=4, f=32)
                                for r in range(4):
                                    nc.vector.transpose(
                                        out=msg[32 * r:32 * r + 32,
                                                off:off + nq, :],
                                        in_=gq[:, :, r, :])
                                off += nq
                        if mode not in ("G", "GT"):
                            for g0 in range(0, tcnt, GRP):
                                cnt = min(GRP, tcnt - g0)
                                nc.vector.tensor_tensor(
                                    out=s[:, g0 * WIN:(g0 + cnt) * WIN],
                                    in0=dst_t[:, t0 + g0:t0 + g0 + cnt, None]
                                        .to_broadcast([128, cnt, WIN]),
                                    in1=io_b[:, 0:cnt * WIN],
                                    op=mybir.AluOpType.is_equal,
                                )

                    resb = rpool.tile([128, BB, OUT_DIM], F32, tag="res")
                    for b in range(b0, b1):
                        acc = ppb.tile([128, OUT_DIM], F32, space="PSUM")
                        tl = bt["btiles"][b]
                        if mode != "full" or not tl:
                            nc.vector.memset(acc[:], 0)
                        else:
                            for i, gi in enumerate(tl):
                                kq = q_of[t0 + gi]
                                rhs = (msg[:, gi, :] if gkind == "sbuf"
                                       else g[:, gi, 32 * kq:32 * kq + OUT_DIM])
                                nc.tensor.matmul(
                                    out=acc[:],
                                    lhsT=s[:, gi * WIN:(gi + 1) * WIN],
                                    rhs=rhs,
                                    start=(i == 0), stop=(i == len(tl) - 1),
                                    skip_group_check=True,
                                )
                        nc.scalar.mul(out=resb[:, b - b0, :], in_=acc[:],
                                      mul=ci_t[:, b:b + 1])
                    nc.sync.dma_start(
                        out=out[b0 * 128:b1 * 128, :]
                            .rearrange("(bb p) f -> p bb f", p=128),
                        in_=resb[:, 0:b1 - b0, :])
    nc.compile()
    return nc


def _in_maps(ins, per_core):
    src_feats = np.ascontiguousarray(np.asarray(ins["src_feats"], dtype=np.float32))
    cj = np.asarray(ins["cj"], dtype=np.float32).reshape(-1)
    ci = np.asarray(ins["ci"], dtype=np.float32).reshape(-1)
    W = np.asarray(ins["W"], dtype=np.float32)
    b = np.asarray(ins["b"], dtype=np.float32).reshape(-1)

    maps = []
    for c in range(N_CORES):
        lo, hi = c * SHARD, (c + 1) * SHARD
        xf = np.zeros((SPAD, IN_DIM), np.float32)
        xf[:SHARD] = src_feats[lo:hi]
        cjf = np.zeros(SPAD, np.float32)
        cjf[:SHARD] = cj[lo:hi]
        cif = np.zeros(SPAD, np.float32)
        cif[:SHARD] = ci[lo:hi]
        m = {
            "xT": np.ascontiguousarray(xf.T),
            "wT": np.ascontiguousarray(W.T),
            "brep": np.tile(b[None, :], (128, 1)),
            "cjT": np.ascontiguousarray(cjf.reshape(NBLK, 128).T),
            "ciT": np.ascontiguousarray(cif.reshape(NBLK, 128).T),
        }
        m.update(per_core[c])
        maps.append(m)
    return maps


def kernel(src_feats, cj, ci, W, b, edge_src, edge_dst):
    ins = {"src_feats": src_feats, "cj": cj, "ci": ci, "W": W, "b": b}
    meta, per_core = _plan(edge_src, edge_dst)
    nc = _build(meta)
    maps = _in_maps(ins, per_core)
    res = run_bass_kernel_spmd(nc, maps, core_ids=list(range(N_CORES)))
    outs = [res.results[c]["out"][:SHARD] for c in range(N_CORES)]
    return np.concatenate(outs, 0).astype(np.float32)



# revision 50
# speedup vs baseline: 1.5810x; 1.5810x over previous
"""GCMC graph-conv kernel for Trainium2, distributed over 8 NeuronCores.

Computes: agg = segment_sum((src_feats @ W.T + b) * cj [edge_src], edge_dst) * ci

Strategy (dst-sharded, one NEFF SPMD on 8 cores):
  - Each core owns 12500 destination nodes and the edges pointing to them.
  - Phase A: each core computes wh = (X_shard @ W.T + b) * cj_shard on the
    TensorEngine and writes it (bf16) into a packed table shard: each 256B row
    holds FOUR nodes' 32-feature messages (node prow -> row prow//4, subcol
    prow%4). Packing keeps dma_gather rows at the required 256B multiple while
    the whole 25088-row table stays addressable by int16 gather indices.
  - AllGather the 8 compact shards (0.8MB each) -> full table in every HBM.
  - Phase B: edges are bucketed by (dst block, q=prow%4, dst half). Each
    128-slot tile gathers its edges' table rows (dma_gather), builds a
    one-hot matrix over its 64-dst window (is_equal on VectorE), and
    scatter-sums via PSUM-accumulating matmuls (rhs = gathered columns
    [32q : 32q+32]). Scale by ci, DMA out.

All control structure (tile counts, windows) is common across the 8 cores
(max over cores); cores pad their slots (dst_shift=-1 kills the one-hot
column; gather idx 0 is harmless).
"""
import sys

if "/opt/trn_rl_repo" not in sys.path:
    sys.path.insert(0, "/opt/trn_rl_repo")

import numpy as np
import ml_dtypes

import concourse.bacc as bacc
import concourse.mybir as mybir
import concourse.tile as tile
from concourse.bass_utils import run_bass_kernel_spmd

# problem constants (hardcoded per harness contract)
N_NODES = 100000
N_EDGES = 1_600_000
IN_DIM = 128
OUT_DIM = 32
N_CORES = 8
SHARD = N_NODES // N_CORES          # 12500 dst nodes per core
NBLK = (SHARD + 127) // 128         # 98 dst blocks per core
SPAD = NBLK * 128                   # 12544 padded shard nodes
TROWS = SPAD * N_CORES // 4         # 25088 packed table rows (4 nodes each)
LROWS = SPAD // 4                   # 3136 packed rows per core shard
ROWELEM = 128                       # bf16 elems per table row = 256B
WIN = 128                           # one-hot window: full block (PSUM base 0)
GRP = 16                            # tiles per is_equal op
BB = 5                              # dst blocks per double-buffered batch
GCAP = 25                           # tiles per dma_gather call
ACH = 14                            # phase-A node blocks per chunk (98 = 7*14)
GKIND = "hbm"                       # gather source: "sbuf" | "hbm"
NSWQ = 2                            # SWDGE queues for gather overlap

F32 = mybir.dt.float32
BF16 = mybir.dt.bfloat16
I16 = mybir.dt.int16


def _plan(edge_src, edge_dst):
    """Pack edges into the common SPMD structure.

    Tiles are ordered per batch, q-major: for q in 0..4, for b in batch,
    the (b, q) bucket's tiles. This makes each batch's same-q tiles
    contiguous so the SBUF-gather path can stream-transpose per q-slab.

    meta:
      ntiles       total tiles
      q_of[t]      table subcolumn (edge prow % 4), global tile index
      batches      list of dicts: t0, tcnt, qn[4], btiles{b: [gi, ...]}
    per core:
      idx  [128, ntiles*8] int16  wrapped packed-row gather indices
      dst  [128, ntiles]   bf16   per-slot dst index in 128-window (-1 = pad)
    """
    src = np.asarray(edge_src).astype(np.int64)
    dst = np.asarray(edge_dst).astype(np.int64)

    core = dst // SHARD
    dst_loc = dst % SHARD
    blk = dst_loc // 128
    dib = dst_loc % 128
    prow = (src // SHARD) * SPAD + (src % SHARD)
    row = prow // 4
    q = prow % 4

    key = ((core * NBLK + blk) * 4 + q)
    order = np.argsort(key, kind="stable")
    s_key, s_dib, s_row = key[order], dib[order], row[order]

    n_cells = N_CORES * NBLK * 4
    bounds = np.searchsorted(s_key, np.arange(n_cells + 1))
    # tiles per (b, q) bucket: max over cores
    ntile_bq = np.zeros((NBLK, 4), np.int64)
    for b in range(NBLK):
        for kq in range(4):
            mx = 0
            for c in range(N_CORES):
                cid = (c * NBLK + b) * 4 + kq
                mx = max(mx, int(bounds[cid + 1] - bounds[cid]))
            ntile_bq[b, kq] = (mx + 127) // 128

    ntiles = 0
    q_of = []
    batches = []
    idx_cols = [[] for _ in range(N_CORES)]
    sh_cols = [[] for _ in range(N_CORES)]

    for b0 in range(0, NBLK, BB):
        b1 = min(b0 + BB, NBLK)
        t0 = ntiles
        qn = []
        btiles = {b: [] for b in range(b0, b1)}
        for kq in range(4):
            nq = 0
            for b in range(b0, b1):
                segs = []
                for c in range(N_CORES):
                    cid = (c * NBLK + b) * 4 + kq
                    segs.append((int(bounds[cid]), int(bounds[cid + 1])))
                for t in range(int(ntile_bq[b, kq])):
                    btiles[b].append(ntiles - t0)
                    ntiles += 1
                    nq += 1
                    q_of.append(kq)
                    for c in range(N_CORES):
                        s, e = segs[c]
                        p = s + t * 128
                        take = max(0, min(e - p, 128))
                        col_i = np.zeros(128, np.int16)
                        col_s = np.full(128, -1.0, np.float32)
                        if take > 0:
                            col_i[:take] = s_row[p:p + take]
                            col_s[:take] = s_dib[p:p + take]
                        idx_cols[c].append(col_i)
                        sh_cols[c].append(col_s)
            qn.append(nq)
        batches.append({"b0": b0, "b1": b1, "t0": t0, "tcnt": ntiles - t0,
                        "qn": qn, "btiles": btiles})

    meta = {"ntiles": ntiles, "q_of": q_of, "batches": batches}

    per_core = []
    for c in range(N_CORES):
        icols = np.stack(idx_cols[c], 0)          # [nt, 128]
        scols = np.stack(sh_cols[c], 0)           # [nt, 128]
        w = icols.reshape(ntiles, 8, 16).transpose(2, 0, 1).reshape(16, ntiles * 8)
        per_core.append({
            "idx": np.tile(w.astype(np.int16), (8, 1)),
            "dst": scols.T.astype(ml_dtypes.bfloat16),
        })
    return meta, per_core


def _phasea_perm():
    """Phase-A node order is identity: tile t, partition p holds local node
    128t + p = packed row index; table write offset is affine (64B * p)."""
    return np.arange(SPAD)


def _build(meta, mode="full", n_devices=N_CORES, no_cc=False, reps=1,
           gkind=GKIND):
    ntiles = meta["ntiles"]
    q_of = meta["q_of"]
    batches = meta["batches"]

    nc = bacc.Bacc("TRN2", target_bir_lowering=False, debug=False,
                   enable_asserts=True, num_devices=n_devices,
                   num_swdge_queues=NSWQ)

    xT = nc.dram_tensor("xT", [128, SPAD * N_CORES], BF16,
                        kind="ExternalInput")
    wT = nc.dram_tensor("wT", [128, OUT_DIM], BF16, kind="ExternalInput")
    brep = nc.dram_tensor("brep", [128, ACH * OUT_DIM], F32,
                          kind="ExternalInput")
    cjT = nc.dram_tensor("cjT", [128, NBLK * N_CORES], F32, kind="ExternalInput")
    ciT = nc.dram_tensor("ciT", [128, NBLK], F32, kind="ExternalInput")
    idx_d = nc.dram_tensor("idx", [128, ntiles * 8], I16, kind="ExternalInput")
    dst_d = nc.dram_tensor("dst", [128, ntiles], BF16, kind="ExternalInput")
    out = nc.dram_tensor("out", [SPAD, OUT_DIM], F32, kind="ExternalOutput")

    gmax = max(bt["tcnt"] for bt in batches)

    with tile.TileContext(nc) as tc:
        with (
            tc.tile_pool(name="dram", bufs=1, space="DRAM") as dram,
            tc.tile_pool(name="const", bufs=1) as cpool,
            tc.tile_pool(name="xa", bufs=2) as xpool,
            tc.tile_pool(name="ha", bufs=4) as hpool,
            tc.tile_pool(name="wa", bufs=2) as wpool,
            tc.tile_pool(name="pa", bufs=4, space="PSUM") as ppa,
            tc.tile_pool(name="gath", bufs=2) as gpool,
            tc.tile_pool(name="idxp", bufs=2) as ipool,
            tc.tile_pool(name="msgp", bufs=2) as mpool,
            tc.tile_pool(name="smat", bufs=2) as spool,
            tc.tile_pool(name="pb", bufs=4, space="PSUM") as ppb,
            tc.tile_pool(name="res", bufs=2) as rpool,
        ):
            table_full = dram.tile([TROWS, ROWELEM], BF16)

            # constants
            wt_t = cpool.tile([128, OUT_DIM], BF16)
            nc.sync.dma_start(out=wt_t[:], in_=wT[:])
            br_t = cpool.tile([128, ACH * OUT_DIM], F32)
            nc.sync.dma_start(out=br_t[:], in_=brep[:])
            bias1 = br_t[0:1, :]                # bias (x ACH) as 1-partition rhs
            ones1 = cpool.tile([1, 128], F32)
            nc.vector.memset(ones1[:], 1.0)
            cj_t = cpool.tile([128, NBLK * N_CORES], F32)
            nc.sync.dma_start(out=cj_t[:], in_=cjT[:])
            ci_t = cpool.tile([128, NBLK], F32)
            nc.sync.dma_start(out=ci_t[:], in_=ciT[:])
            dst_t = cpool.tile([128, ntiles], BF16)
            nc.sync.dma_start(out=dst_t[:], in_=dst_d[:])
            # iota: [128, GRP*WIN] bf16, value = col % WIN
            io_i = cpool.tile([128, GRP * WIN], I16)
            nc.gpsimd.iota(io_i[:], pattern=[[0, GRP], [1, WIN]], base=0,
                           channel_multiplier=0)
            io_b = cpool.tile([128, GRP * WIN], BF16)
            nc.vector.tensor_copy(out=io_b[:], in_=io_i[:])
            gsem = nc.alloc_semaphore("gsem")
            if gkind == "sbuf":
                tbl_sb = cpool.tile([128, TROWS // 128, ROWELEM], BF16)

            # packed-table write view: chunk c, partition p, block j, feat f
            # -> DRAM offset 8192*(ACH*c + j) + 64*p + 2*f  (bytes; affine)
            tab_w = table_full[:].rearrange("(c j r) (q f) -> c (r q) j f",
                                            j=ACH, r=32, q=4)

            for _rep in range(reps):
                # ---- Phase A (replicated): full wh table computed locally ----
                ntile_a = SPAD * N_CORES // 128  # 784
                nchunk = ntile_a // ACH
                for c in range(nchunk):
                    a0 = c * ACH
                    xt = xpool.tile([128, ACH * 128], BF16)
                    nc.sync.dma_start(out=xt[:],
                                      in_=xT[:, a0 * 128:(a0 + ACH) * 128])
                    ph = ppa.tile([128, ACH, OUT_DIM], F32, space="PSUM")
                    # bias via one rank-1 accumulating matmul over the chunk
                    nc.tensor.matmul(out=ph[:].rearrange("p j f -> p (j f)"),
                                     lhsT=ones1[:], rhs=bias1,
                                     start=True, stop=False,
                                     skip_group_check=True)
                    for j in range(ACH):
                        nc.tensor.matmul(out=ph[:, j, :],
                                         lhsT=xt[:, j * 128:(j + 1) * 128],
                                         rhs=wt_t[:], start=False, stop=True,
                                         skip_group_check=True)
                    wh = wpool.tile([128, ACH, OUT_DIM], BF16)
                    nc.vector.tensor_tensor(
                        out=wh[:],
                        in0=ph[:],
                        in1=cj_t[:, a0:a0 + ACH, None]
                            .to_broadcast([128, ACH, OUT_DIM]),
                        op=mybir.AluOpType.mult,
                    )
                    nc.sync.dma_start(out=tab_w[c], in_=wh[:])

                # ---- table DRAM -> SBUF stripes (SBUF-source gather) ----
                if gkind == "sbuf" and mode not in ("A", "AG"):
                    nc.sync.dma_start(
                        out=tbl_sb[:],
                        in_=table_full[:].rearrange("(s p) f -> p s f", p=128))

                # ---- Phase B ----
                for bt in batches:
                    b0, b1, t0, tcnt = bt["b0"], bt["b1"], bt["t0"], bt["tcnt"]
                    s = spool.tile([128, gmax * WIN], BF16, tag="s")
                    if gkind == "sbuf":
                        g = gpool.tile([128, 1, gmax * 128], BF16, tag="g")
                        msg = mpool.tile([128, gmax, OUT_DIM], BF16, tag="m")
                    else:
                        g = gpool.tile([128, gmax, ROWELEM], BF16, tag="g")
                    if mode not in ("A", "AG"):
                        idx_t = ipool.tile([128, gmax * 8], I16, tag="i")
                        nc.sync.dma_start(
                            out=idx_t[:, 0:tcnt * 8],
                            in_=idx_d[:, t0 * 8:(t0 + tcnt) * 8])
                        for c0 in range(0, tcnt, GCAP):
                            cn = min(GCAP, tcnt - c0)
                            isl = idx_t[:, c0 * 8:(c0 + cn) * 8]
                            if gkind == "sbuf":
                                nc.gpsimd.dma_gather(
                                    out_ap=g[:, :, c0 * 128:(c0 + cn) * 128],
                                    in_ap=tbl_sb[:].rearrange("p s f -> p (s f)"),
                                    idxs_ap=isl,
                                    num_idxs=cn * 128, num_idxs_reg=cn * 128,
                                    elem_size=ROWELEM, transpose=True,
                                    sbuf_tokens_per_rank=128,
                                    sbuf_free_dim_per_rank=2 * ROWELEM,
                                )
                            else:
                                nc.gpsimd.dma_gather(
                                    out_ap=g[:, c0:c0 + cn, :],
                                    in_ap=table_full[:],
                                    idxs_ap=isl,
                                    num_idxs=cn * 128, num_idxs_reg=cn * 128,
                                    elem_size=ROWELEM, single_packet=False,
                                    queue_num=(c0 // GCAP) % NSWQ,
                                )
                        if gkind == "sbuf" and mode not in ("G",):
                            # per-q stream-transpose: msgT [32, E] -> msg [E, 32]
                            off = 0
                            for kq in range(4):
                                nq = bt["qn"][kq]
                                if nq == 0:
                                    continue
                                gq = g[32 * kq:32 * kq + 32, 0,
                                       off * 128:(off + nq) * 128] \
                                    .rearrange("p (t r f) -> p t r f", r=4, f=32)
                                for r in range(4):
                                    nc.vector.transpose(
                                        out=msg[32 * r:32 * r + 32,
                                                off:off + nq, :],
                                        in_=gq[:, :, r, :])
                                off += nq
                        if mode not in ("G", "GT"):
                            for g0 in range(0, tcnt, GRP):
                                cnt = min(GRP, tcnt - g0)
                                nc.vector.tensor_tensor(
                                    out=s[:, g0 * WIN:(g0 + cnt) * WIN],
                                    in0=dst_t[:, t0 + g0:t0 + g0 + cnt, None]
                                        .to_broadcast([128, cnt, WIN]),
                                    in1=io_b[:, 0:cnt * WIN],
                                    op=mybir.AluOpType.is_equal,
                                )

                    resb = rpool.tile([128, BB, OUT_DIM], F32, tag="res")
                    for b in range(b0, b1):
                        acc = ppb.tile([128, OUT_DIM], F32, space="PSUM")
                        tl = bt["btiles"][b]
                        if mode != "full" or not tl:
                            nc.vector.memset(acc[:], 0)
                        else:
                            for i, gi in enumerate(tl):
                                kq = q_of[t0 + gi]
                                rhs = (msg[:, gi, :] if gkind == "sbuf"
                                       else g[:, gi, 32 * kq:32 * kq + OUT_DIM])
                                nc.tensor.matmul(
                                    out=acc[:],
                                    lhsT=s[:, gi * WIN:(gi + 1) * WIN],
                                    rhs=rhs,
                                    start=(i == 0), stop=(i == len(tl) - 1),
                                    skip_group_check=True,
                                )
                        nc.scalar.mul(out=resb[:, b - b0, :], in_=acc[:],
                                      mul=ci_t[:, b:b + 1])
                    nc.sync.dma_start(
                        out=out[b0 * 128:b1 * 128, :]
                            .rearrange("(bb p) f -> p bb f", p=128),
                        in_=resb[:, 0:b1 - b0, :])
    nc.compile()
    return nc


def _in_maps(ins, per_core):
    src_feats = np.ascontiguousarray(np.asarray(ins["src_feats"], dtype=np.float32))
    cj = np.asarray(ins["cj"], dtype=np.float32).reshape(-1)
    ci = np.asarray(ins["ci"], dtype=np.float32).reshape(-1)
    W = np.asarray(ins["W"], dtype=np.float32)
    b = np.asarray(ins["b"], dtype=np.float32).reshape(-1)

    # replicated phase A: every core gets the FULL padded node table
    xf = np.zeros((SPAD * N_CORES, IN_DIM), np.float32)
    cjf = np.zeros(SPAD * N_CORES, np.float32)
    for c in range(N_CORES):
        lo, hi = c * SHARD, (c + 1) * SHARD
        xf[c * SPAD:c * SPAD + SHARD] = src_feats[lo:hi]
        cjf[c * SPAD:c * SPAD + SHARD] = cj[lo:hi]
    xT = np.ascontiguousarray(xf.T).astype(ml_dtypes.bfloat16)
    cjT = np.ascontiguousarray(cjf.reshape(NBLK * N_CORES, 128).T)
    brep = np.tile(b[None, :], (128, ACH))
    wTc = np.ascontiguousarray(W.T).astype(ml_dtypes.bfloat16)

    maps = []
    for c in range(N_CORES):
        lo, hi = c * SHARD, (c + 1) * SHARD
        cif = np.zeros(SPAD, np.float32)
        cif[:SHARD] = ci[lo:hi]
        m = {
            "xT": xT,
            "wT": wTc,
            "brep": brep,
            "cjT": cjT,
            "ciT": np.ascontiguousarray(cif.reshape(NBLK, 128).T),
        }
        m.update(per_core[c])
        maps.append(m)
    return maps


def kernel(src_feats, cj, ci, W, b, edge_src, edge_dst):
    ins = {"src_feats": src_feats, "cj": cj, "ci": ci, "W": W, "b": b}
    meta, per_core = _plan(edge_src, edge_dst)
    nc = _build(meta)
    maps = _in_maps(ins, per_core)
    res = run_bass_kernel_spmd(nc, maps, core_ids=list(range(N_CORES)))
    outs = [res.results[c]["out"][:SHARD] for c in range(N_CORES)]
    return np.concatenate(outs, 0).astype(np.float32)

